# revision 2
# baseline (speedup 1.0000x reference)
"""AxialAttention Trainium2 kernel (v3: single-core, For_i over batches).

Problem: x [8, 256, 128, 128]; 1x1-conv q/k/v projections (8 heads, head_dim 32),
axial (row + column) softmax attention, output projection, residual.

Strategy: the metric is the marginal per-call cost of a queued execution
stream through the axon tunnel. Multi-device (shard_map) dispatch costs
~3-5 ms per call regardless of I/O bytes, while single-device execution
pipelines to ~zero marginal overhead. So all 8 batches run on ONE core,
sequenced by a For_i hardware loop; the metric becomes pure device time.

Per-batch body (identical math to v2):
- Inputs: x fp8-e4m3 (residual is reconstructed host-side from f32 x),
  packed transposed bf16 weights, biases.
- q,k projected once into resident SBUF tensors (bias folded into q only:
  softmax drops q.bk and bq.bk logit terms; v-bias folds into a host-side
  constant since attention weights sum to 1).
- Vertical (column) attention first writes o_v (resident fp8), horizontal
  attention adds it per row block, applies Wo, stores int8 (scale 64).
  Host adds x + (Wo@(2 bv)+bo) and rescales.
"""
import numpy as np
import ml_dtypes
from contextlib import ExitStack

import concourse.bass as bass
import concourse.bacc as bacc
import concourse.tile as tile
from concourse import mybir
from concourse.bass import ds
from concourse.bass_utils import run_bass_kernel_spmd

B, C, H, W = 8, 256, 128, 128
NH, HD = 8, 32          # heads, head dim
CH = 2                  # channel chunks of 128
LB = 8                  # lines per block
SCALE = HD ** -0.5
BF16 = mybir.dt.bfloat16
F32 = mybir.dt.float32
FP8 = mybir.dt.float8e4
I8 = mybir.dt.int8
OUT_SCALE = 64.0

_CACHE = {}


def build_nc(n_lines=H, lb=LB, n_batches=B):
    """Build + compile the single-core Bass module covering n_batches batches.
    n_lines<H builds a reduced variant (first n_lines rows/cols attended) for
    fast simulation; q/k are always projected for the full image so reduced
    outputs stay exact."""
    nc = bacc.Bacc("TRN2", target_bir_lowering=False, debug=False)

    x_h = nc.dram_tensor("x", [n_batches, C, H, W], FP8, kind="ExternalInput")
    w_h = nc.dram_tensor("wpack", [C, 4 * C], BF16, kind="ExternalInput")
    b_h = nc.dram_tensor("bvec", [C, 2], F32, kind="ExternalInput")
    out_h = nc.dram_tensor("out", [n_batches, CH, 128, H * W], I8,
                           kind="ExternalOutput")

    with tile.TileContext(nc) as tc, ExitStack() as ctx:
        const = ctx.enter_context(tc.tile_pool(name="const", bufs=1))
        data = ctx.enter_context(tc.tile_pool(name="data", bufs=1))
        sbv = ctx.enter_context(tc.tile_pool(name="sbv", bufs=2))
        sbe = ctx.enter_context(tc.tile_pool(name="sbe", bufs=2))
        sbo = ctx.enter_context(tc.tile_pool(name="sbo", bufs=2))
        pss = ctx.enter_context(tc.tile_pool(name="pss", bufs=1, space="PSUM"))
        psz = ctx.enter_context(tc.tile_pool(name="psz", bufs=2, space="PSUM"))
        psp = ctx.enter_context(tc.tile_pool(name="psp", bufs=2, space="PSUM"))

        # --- constants (loaded once, outside the batch loop) ---
        # wpack = [Wq^T | Wk^T | Wv^T | Wo^T], each [c_in, c_out]
        wts = []
        for i, name in enumerate(["wq", "wk", "wv", "wo"]):
            wt = const.tile([128, CH, CH, 128], BF16, tag=name)
            nc.sync.dma_start(
                wt[:], w_h[:, i * C:(i + 1) * C].rearrange(
                    "(cc p) (co q) -> p cc co q", p=128, q=128))
            wts.append(wt)
        wq, wk, wv, wo = wts
        bqt = const.tile([128, CH], F32, tag="bq")
        nc.sync.dma_start(bqt[:], b_h[:, 0].rearrange("(cc p) -> p cc", p=128))
        ones = const.tile([128, HD], BF16, tag="ones")
        nc.vector.memset(ones[:], 1.0)

        with tc.For_i(0, n_batches) as bi:
            # --- resident tensors (rebuilt per batch) ---
            x_sb = data.tile([128, CH, H * W], FP8, tag="x")
            nc.sync.dma_start(
                x_sb[:], x_h[ds(bi, 1)].rearrange(
                    "b (cc p) y w -> p (b cc) (y w)", p=128))
            q_sb = data.tile([128, CH, H * W], BF16, tag="q")
            k_sb = data.tile([128, CH, H * W], FP8, tag="k")
            o_v = data.tile([128, CH, H, W], FP8, tag="ov")
            if n_lines < H:
                # reduced sim build: phases only write the first n_lines columns
                nc.vector.memset(o_v[:], 0.0)

            x4 = x_sb[:].rearrange("p g (y w) -> p g y w", w=W)
            q4 = q_sb[:].rearrange("p g (y w) -> p g y w", w=W)
            k4 = k_sb[:].rearrange("p g (y w) -> p g y w", w=W)

            # --- phase 0: q,k projections (full image, resident) ---
            for blk in range(H // lb):
                for co in range(CH):
                    for nb in range(2):
                        ns = slice(blk * lb * W + nb * 512,
                                   blk * lb * W + (nb + 1) * 512)
                        qp = psp.tile([128, 512], F32, tag="pp")
                        for cc in range(CH):
                            nc.tensor.matmul(qp[:], wq[:, cc, co, :],
                                             x_sb[:, cc, ns],
                                             start=(cc == 0), stop=(cc == CH - 1))
                        # ACT is idle during this phase; bias-add there
                        nc.scalar.activation(q_sb[:, co, ns], qp[:],
                                             mybir.ActivationFunctionType.Identity,
                                             bias=bqt[:, co:co + 1])
                        kp = psp.tile([128, 512], F32, tag="pp")
                        for cc in range(CH):
                            nc.tensor.matmul(kp[:], wk[:, cc, co, :],
                                             x_sb[:, cc, ns],
                                             start=(cc == 0), stop=(cc == CH - 1))
                        nc.vector.tensor_copy(k_sb[:, co, ns], kp[:])

            # --- phases: axis 1 = vertical (first), axis 0 = horizontal ---
            for axis in (1, 0):
                for blk in range(n_lines // lb):
                    l0 = blk * lb
                    # transposed-v for this block's lines: vt[spatial, line, c]
                    vt = sbv.tile([128, lb, C], BF16, tag="vt")
                    for lp2 in range(lb // 2):
                        vp = psp.tile([128, 2, C], F32, tag="pp")
                        for i in range(2):
                            l = lp2 * 2 + i
                            for cc in range(CH):
                                xop = (x4[:, cc, l0 + l, :] if axis == 0
                                       else x4[:, cc, :, l0 + l])
                                nc.tensor.matmul(vp[:, i, :], xop, wv[:, cc, :],
                                                 start=(cc == 0),
                                                 stop=(cc == CH - 1))
                        nc.vector.tensor_copy(vt[:, lp2 * 2:lp2 * 2 + 2, :], vp[:])

                    if axis == 0:
                        t_ob = sbo.tile([128, CH, lb, W], BF16, tag="tob")

                    # attention, in line pairs (S for pair: 4 psum banks)
                    for lp in range(lb // 2):
                        s4 = pss.tile([128, 4, 4, W], F32, tag="s")
                        e4 = sbe.tile([128, 4, 4, W], BF16, tag="e")
                        for p in range(2):
                            line = l0 + lp * 2 + p
                            for h in range(NH):
                                j, g = h % 4, h // 4
                                if axis == 0:
                                    ls = slice(line * W, (line + 1) * W)
                                    kop = k_sb[j * 32:(j + 1) * 32, g, ls]
                                    qop = q_sb[j * 32:(j + 1) * 32, g, ls]
                                else:
                                    kop = k4[j * 32:(j + 1) * 32, g, :, line]
                                    qop = q4[j * 32:(j + 1) * 32, g, :, line]
                                nc.tensor.matmul(s4[:, j, p * 2 + g, :], kop, qop,
                                                 start=True, stop=True,
                                                 tile_position=(j * 32, 0))
                            # per-line exp over strided slots: exp(line p)
                            # overlaps S matmuls of line p+1 and AV of line p-1
                            nc.scalar.activation(e4[:, :, p * 2:p * 2 + 2, :],
                                                 s4[:, :, p * 2:p * 2 + 2, :],
                                                 mybir.ActivationFunctionType.Exp,
                                                 scale=SCALE)
                        for p in range(2):
                            l = lp * 2 + p
                            line = l0 + l
                            oz = psz.tile([128, 4, W], F32, tag="oz")
                            for h in range(NH):
                                j, g = h % 4, h // 4
                                nc.tensor.matmul(oz[j * 32:(j + 1) * 32, g, :],
                                                 vt[:, l, h * HD:(h + 1) * HD],
                                                 e4[:, j, p * 2 + g, :],
                                                 start=True, stop=True,
                                                 tile_position=(0, j * 32))
                            for j in range(4):
                                nc.tensor.matmul(oz[j * 32:(j + 1) * 32, 2:4, :],
                                                 ones[:],
                                                 e4[:, j, p * 2:p * 2 + 2, :],
                                                 start=True, stop=True,
                                                 tile_position=(0, j * 32))
                            zr = sbe.tile([128, CH, W], F32, tag="zr")
                            nc.vector.reciprocal(zr[:], oz[:, 2:4, :])
                            dst = (t_ob[:, :, l, :] if axis == 0
                                   else o_v[:, :, :, line])
                            nc.vector.tensor_tensor(dst, oz[:, 0:2, :], zr[:],
                                                    op=mybir.AluOpType.mult)

                    if axis == 0:
                        # merge with vertical output, project Wo, store int8
                        ob2 = sbo.tile([128, CH, lb, W], BF16, tag="ob2")
                        nc.vector.tensor_tensor(ob2[:], t_ob[:],
                                                o_v[:, :, l0:l0 + lb, :],
                                                op=mybir.AluOpType.add)
                        pt = sbo.tile([128, CH, lb * W], I8, tag="pt")
                        for co in range(CH):
                            for nb in range(2):
                                pp = psp.tile([128, 512], F32, tag="pp")
                                lsl = slice(nb * 4, (nb + 1) * 4)
                                for cc in range(CH):
                                    nc.tensor.matmul(pp[:], wo[:, cc, co, :],
                                                     ob2[:, cc, lsl, :],
                                                     start=(cc == 0),
                                                     stop=(cc == CH - 1))
                                nc.vector.tensor_scalar_mul(
                                    pt[:, co, nb * 512:(nb + 1) * 512], pp[:],
                                    OUT_SCALE)
                        nc.sync.dma_start(
                            out_h[ds(bi, 1), :, :, l0 * W:(l0 + lb) * W].rearrange(
                                "b co p s -> p (b co) s"),
                            pt[:])

    nc.compile()
    return nc


def _get_nc():
    if "nc" not in _CACHE:
        _CACHE["nc"] = build_nc()
    return _CACHE["nc"]


def make_inputs(x, Wq, bq, Wk, Wv, Wo):
    """Single-core input map (host-side prep)."""
    xbf = x.astype(ml_dtypes.float8_e4m3)
    wpack = np.concatenate(
        [np.ascontiguousarray(Wq.T), np.ascontiguousarray(Wk.T),
         np.ascontiguousarray(Wv.T), np.ascontiguousarray(Wo.T)],
        axis=1).astype(ml_dtypes.bfloat16)
    bvec = np.stack([bq, np.zeros_like(bq)], axis=1).astype(np.float32)
    return [dict(wpack=wpack, bvec=bvec, x=xbf)]


def kernel(x, Wq, bq, Wk, bk, Wv, bv, Wo, bo):
    x = np.asarray(x, np.float32)
    Wq, bq = np.asarray(Wq, np.float32), np.asarray(bq, np.float32)
    Wk = np.asarray(Wk, np.float32)
    Wv, bv = np.asarray(Wv, np.float32), np.asarray(bv, np.float32)
    Wo, bo = np.asarray(Wo, np.float32), np.asarray(bo, np.float32)

    nc = _get_nc()
    in_maps = make_inputs(x, Wq, bq, Wk, Wv, Wo)
    res = run_bass_kernel_spmd(nc, in_maps, [0])

    cvec = (Wo @ (2.0 * bv) + bo).astype(np.float32)
    o = res.results[0]["out"].astype(np.float32).reshape(B, C, H, W)
    o *= 1.0 / OUT_SCALE
    o += cvec[None, :, None, None]
    o += x
    return o


# revision 22
# speedup vs baseline: 1.0386x; 1.0386x over previous
"""AxialAttention Trainium2 kernel (v3: single-core, For_i over batches).

Problem: x [8, 256, 128, 128]; 1x1-conv q/k/v projections (8 heads, head_dim 32),
axial (row + column) softmax attention, output projection, residual.

Strategy: the metric is the marginal per-call cost of a queued execution
stream through the axon tunnel. Multi-device (shard_map) dispatch costs
~3-5 ms per call regardless of I/O bytes, while single-device execution
pipelines to ~zero marginal overhead. So all 8 batches run on ONE core,
sequenced by a For_i hardware loop; the metric becomes pure device time.

Per-batch body (identical math to v2):
- Inputs: x fp8-e4m3 (residual is reconstructed host-side from f32 x),
  packed transposed bf16 weights, biases.
- q,k projected once into resident SBUF tensors (bias folded into q only:
  softmax drops q.bk and bq.bk logit terms; v-bias folds into a host-side
  constant since attention weights sum to 1).
- Vertical (column) attention first writes o_v (resident fp8), horizontal
  attention adds it per row block, applies Wo, stores int8 (scale 64).
  Host adds x + (Wo@(2 bv)+bo) and rescales.
"""
import numpy as np
import ml_dtypes
from contextlib import ExitStack

import concourse.bass as bass
import concourse.bacc as bacc
import concourse.tile as tile
from concourse import mybir
from concourse.bass import ds
from concourse.bass_utils import run_bass_kernel_spmd

B, C, H, W = 8, 256, 128, 128
NH, HD = 8, 32          # heads, head dim
CH = 2                  # channel chunks of 128
LB = 8                  # lines per block
SCALE = HD ** -0.5
BF16 = mybir.dt.bfloat16
F32 = mybir.dt.float32
FP8 = mybir.dt.float8e4
I8 = mybir.dt.int8
OUT_SCALE = 64.0
FP8_W = False           # fp8 weights + DoubleRow projections

_CACHE = {}


def build_nc(n_lines=H, lb=LB, n_batches=B, unroll=B, fp8_w=None):
    """Build + compile the single-core Bass module covering n_batches batches.
    n_lines<H builds a reduced variant (first n_lines rows/cols attended) for
    fast simulation; q/k are always projected for the full image so reduced
    outputs stay exact. `unroll` python-unrolls that many batches per For_i
    iteration (fewer all-engine barriers, cross-batch overlap at seams);
    unroll >= n_batches builds with no hardware loop at all."""
    nc = bacc.Bacc("TRN2", target_bir_lowering=False, debug=False)

    if fp8_w is None:
        fp8_w = FP8_W
    WDT = FP8 if fp8_w else BF16
    DR = mybir.MatmulPerfMode.DoubleRow
    x_h = nc.dram_tensor("x", [n_batches, C, H, W], FP8, kind="ExternalInput")
    w_h = nc.dram_tensor("wpack", [C, 4 * C], WDT, kind="ExternalInput")
    b_h = nc.dram_tensor("bvec", [C, 2], F32, kind="ExternalInput")
    out_h = nc.dram_tensor("out", [n_batches, CH, 128, H * W], I8,
                           kind="ExternalOutput")

    with tile.TileContext(nc) as tc, ExitStack() as ctx:
        const = ctx.enter_context(tc.tile_pool(name="const", bufs=1))
        data = ctx.enter_context(tc.tile_pool(name="data", bufs=1))
        sbv = ctx.enter_context(tc.tile_pool(name="sbv", bufs=2))
        sbe = ctx.enter_context(tc.tile_pool(name="sbe", bufs=2))
        sbo = ctx.enter_context(tc.tile_pool(name="sbo", bufs=2))
        pss = ctx.enter_context(tc.tile_pool(name="pss", bufs=1, space="PSUM"))
        psz = ctx.enter_context(tc.tile_pool(name="psz", bufs=2, space="PSUM"))
        psp = ctx.enter_context(tc.tile_pool(name="psp", bufs=2, space="PSUM"))

        # --- constants (loaded once, outside the batch loop) ---
        # wpack = [Wq^T | Wk^T | Wv^T | Wo^T], each [c_in, c_out]
        wts = []
        for i, name in enumerate(["wq", "wk", "wv", "wo"]):
            wt = const.tile([128, CH, CH, 128], WDT, tag=name)
            nc.sync.dma_start(
                wt[:], w_h[:, i * C:(i + 1) * C].rearrange(
                    "(cc p) (co q) -> p cc co q", p=128, q=128))
            wts.append(wt)
        wq, wk, wv, wo = wts
        bqt = const.tile([128, CH], F32, tag="bq")
        nc.sync.dma_start(bqt[:], b_h[:, 0].rearrange("(cc p) -> p cc", p=128))
        ones = const.tile([128, HD], BF16, tag="ones")
        nc.vector.memset(ones[:], 1.0)

        def batch_body(bsel):
            # --- resident tensors (rebuilt per batch) ---
            x_sb = data.tile([128, CH, H * W], FP8, tag="x")
            nc.sync.dma_start(
                x_sb[:], x_h[bsel].rearrange(
                    "b (cc p) y w -> p (b cc) (y w)", p=128))
            q_sb = data.tile([128, CH, H * W], BF16, tag="q")
            k_sb = data.tile([128, CH, H * W], FP8, tag="k")
            o_v = data.tile([128, CH, H, W], FP8, tag="ov")
            if n_lines < H:
                # reduced sim build: phases only write the first n_lines columns
                nc.vector.memset(o_v[:], 0.0)

            x4 = x_sb[:].rearrange("p g (y w) -> p g y w", w=W)
            q4 = q_sb[:].rearrange("p g (y w) -> p g y w", w=W)
            k4 = k_sb[:].rearrange("p g (y w) -> p g y w", w=W)

            # --- phase 0: q,k projections (full image, resident) ---
            for blk in range(H // lb):
                for co in range(CH):
                    for nb in range(2):
                        ns = slice(blk * lb * W + nb * 512,
                                   blk * lb * W + (nb + 1) * 512)
                        qp = psp.tile([128, 512], F32, tag="pp")
                        if fp8_w:
                            nc.tensor.matmul(qp[:], wq[:, :, co, :],
                                             x_sb[:, :, ns], perf_mode=DR,
                                             start=True, stop=True)
                        else:
                            for cc in range(CH):
                                nc.tensor.matmul(qp[:], wq[:, cc, co, :],
                                                 x_sb[:, cc, ns],
                                                 start=(cc == 0),
                                                 stop=(cc == CH - 1))
                        # ACT is idle during this phase; bias-add there
                        nc.scalar.activation(q_sb[:, co, ns], qp[:],
                                             mybir.ActivationFunctionType.Identity,
                                             bias=bqt[:, co:co + 1])
                        kp = psp.tile([128, 512], F32, tag="pp")
                        if fp8_w:
                            nc.tensor.matmul(kp[:], wk[:, :, co, :],
                                             x_sb[:, :, ns], perf_mode=DR,
                                             start=True, stop=True)
                        else:
                            for cc in range(CH):
                                nc.tensor.matmul(kp[:], wk[:, cc, co, :],
                                                 x_sb[:, cc, ns],
                                                 start=(cc == 0),
                                                 stop=(cc == CH - 1))
                        nc.vector.tensor_copy(k_sb[:, co, ns], kp[:])

            # --- phases: axis 1 = vertical (first), axis 0 = horizontal ---
            for axis in (1, 0):
                for blk in range(n_lines // lb):
                    l0 = blk * lb
                    # transposed-v for this block's lines: vt[spatial, line, c]
                    vt = sbv.tile([128, lb, C], BF16, tag="vt")
                    for lp2 in range(lb // 2):
                        vp = psp.tile([128, 2, C], F32, tag="pp")
                        for i in range(2):
                            l = lp2 * 2 + i
                            if fp8_w:
                                xop = (x4[:, :, l0 + l, :] if axis == 0
                                       else x4[:, :, :, l0 + l])
                                nc.tensor.matmul(vp[:, i, :], xop,
                                                 wv[:].rearrange(
                                                     "p cc co q -> p cc (co q)"),
                                                 perf_mode=DR,
                                                 start=True, stop=True)
                            else:
                                for cc in range(CH):
                                    xop = (x4[:, cc, l0 + l, :] if axis == 0
                                           else x4[:, cc, :, l0 + l])
                                    nc.tensor.matmul(vp[:, i, :], xop,
                                                     wv[:, cc, :],
                                                     start=(cc == 0),
                                                     stop=(cc == CH - 1))
                        nc.vector.tensor_copy(vt[:, lp2 * 2:lp2 * 2 + 2, :], vp[:])

                    if axis == 0:
                        t_ob = sbo.tile([128, CH, lb, W], BF16, tag="tob")

                    # attention, in line pairs (S for pair: 4 psum banks)
                    for lp in range(lb // 2):
                        s4 = pss.tile([128, 4, 4, W], F32, tag="s")
                        e4 = sbe.tile([128, 4, 4, W], BF16, tag="e")
                        for p in range(2):
                            line = l0 + lp * 2 + p
                            for h in range(NH):
                                j, g = h % 4, h // 4
                                if axis == 0:
                                    ls = slice(line * W, (line + 1) * W)
                                    kop = k_sb[j * 32:(j + 1) * 32, g, ls]
                                    qop = q_sb[j * 32:(j + 1) * 32, g, ls]
                                else:
                                    kop = k4[j * 32:(j + 1) * 32, g, :, line]
                                    qop = q4[j * 32:(j + 1) * 32, g, :, line]
                                nc.tensor.matmul(s4[:, j, p * 2 + g, :], kop, qop,
                                                 start=True, stop=True,
                                                 tile_position=(j * 32, 0))
                            # per-line exp over strided slots: exp(line p)
                            # overlaps S matmuls of line p+1 and AV of line p-1
                            nc.scalar.activation(e4[:, :, p * 2:p * 2 + 2, :],
                                                 s4[:, :, p * 2:p * 2 + 2, :],
                                                 mybir.ActivationFunctionType.Exp,
                                                 scale=SCALE)
                        for p in range(2):
                            l = lp * 2 + p
                            line = l0 + l
                            oz = psz.tile([128, 4, W], F32, tag="oz")
                            for h in range(NH):
                                j, g = h % 4, h // 4
                                nc.tensor.matmul(oz[j * 32:(j + 1) * 32, g, :],
                                                 vt[:, l, h * HD:(h + 1) * HD],
                                                 e4[:, j, p * 2 + g, :],
                                                 start=True, stop=True,
                                                 tile_position=(0, j * 32))
                            for j in range(4):
                                nc.tensor.matmul(oz[j * 32:(j + 1) * 32, 2:4, :],
                                                 ones[:],
                                                 e4[:, j, p * 2:p * 2 + 2, :],
                                                 start=True, stop=True,
                                                 tile_position=(0, j * 32))
                            zr = sbe.tile([128, CH, W], F32, tag="zr")
                            nc.vector.reciprocal(zr[:], oz[:, 2:4, :])
                            dst = (t_ob[:, :, l, :] if axis == 0
                                   else o_v[:, :, :, line])
                            nc.vector.tensor_tensor(dst, oz[:, 0:2, :], zr[:],
                                                    op=mybir.AluOpType.mult)

                    if axis == 0:
                        # merge with vertical output, project Wo, store int8
                        ob2 = sbo.tile([128, CH, lb, W], FP8 if fp8_w else BF16,
                                       tag="ob2")
                        nc.vector.tensor_tensor(ob2[:], t_ob[:],
                                                o_v[:, :, l0:l0 + lb, :],
                                                op=mybir.AluOpType.add)
                        pt = sbo.tile([128, CH, lb * W], I8, tag="pt")
                        for co in range(CH):
                            for nb in range(2):
                                pp = psp.tile([128, 512], F32, tag="pp")
                                lsl = slice(nb * 4, (nb + 1) * 4)
                                if fp8_w:
                                    nc.tensor.matmul(pp[:], wo[:, :, co, :],
                                                     ob2[:, :, lsl, :],
                                                     perf_mode=DR,
                                                     start=True, stop=True)
                                else:
                                    for cc in range(CH):
                                        nc.tensor.matmul(pp[:], wo[:, cc, co, :],
                                                         ob2[:, cc, lsl, :],
                                                         start=(cc == 0),
                                                         stop=(cc == CH - 1))
                                nc.vector.tensor_scalar_mul(
                                    pt[:, co, nb * 512:(nb + 1) * 512], pp[:],
                                    OUT_SCALE)
                        nc.sync.dma_start(
                            out_h[bsel, :, :, l0 * W:(l0 + lb) * W].rearrange(
                                "b co p s -> p (b co) s"),
                            pt[:])

        if unroll >= n_batches:
            for b in range(n_batches):
                batch_body(ds(b, 1))
        else:
            assert n_batches % unroll == 0
            with tc.For_i(0, n_batches, unroll) as bi:
                for u in range(unroll):
                    batch_body(ds(bi + u, 1))

    nc.compile()
    return nc


def _get_nc():
    if "nc" not in _CACHE:
        _CACHE["nc"] = build_nc()
    return _CACHE["nc"]


def make_inputs(x, Wq, bq, Wk, Wv, Wo):
    """Single-core input map (host-side prep)."""
    xbf = x.astype(ml_dtypes.float8_e4m3)
    wpack = np.concatenate(
        [np.ascontiguousarray(Wq.T), np.ascontiguousarray(Wk.T),
         np.ascontiguousarray(Wv.T), np.ascontiguousarray(Wo.T)],
        axis=1).astype(ml_dtypes.float8_e4m3 if FP8_W else ml_dtypes.bfloat16)
    bvec = np.stack([bq, np.zeros_like(bq)], axis=1).astype(np.float32)
    return [dict(wpack=wpack, bvec=bvec, x=xbf)]


def kernel(x, Wq, bq, Wk, bk, Wv, bv, Wo, bo):
    x = np.asarray(x, np.float32)
    Wq, bq = np.asarray(Wq, np.float32), np.asarray(bq, np.float32)
    Wk = np.asarray(Wk, np.float32)
    Wv, bv = np.asarray(Wv, np.float32), np.asarray(bv, np.float32)
    Wo, bo = np.asarray(Wo, np.float32), np.asarray(bo, np.float32)

    nc = _get_nc()
    in_maps = make_inputs(x, Wq, bq, Wk, Wv, Wo)
    res = run_bass_kernel_spmd(nc, in_maps, [0])

    cvec = (Wo @ (2.0 * bv) + bo).astype(np.float32)
    o = res.results[0]["out"].astype(np.float32).reshape(B, C, H, W)
    o *= 1.0 / OUT_SCALE
    o += cvec[None, :, None, None]
    o += x
    return o
